# revision 1
# baseline (speedup 1.0000x reference)
"""AttentiveFP GNN (nn_AFP_jittable) as a distributed Bass kernel on 8 TRN2
NeuronCores.

Sharding: molecules are split across the 8 cores; nodes at molecule
boundaries (padded to NP); each edge is owned by the core owning its dst
node.  Edges are sorted by dst and grouped into 128-node windows with a
uniform tile budget T per window (SPMD-uniform shapes).  Per GAT layer:
node-phase matmuls run in transposed [k, n] layout, payload rows are
PE-transposed into a local DRAM table, AllGathered, then per-edge 512B-row
indirect-DMA gathers (one 128-row gather per tile) feed one-hot scatter
matmuls that accumulate [node_window, H+1] (messages + softmax denominator)
in PSUM.  Segment softmax skips max-subtraction (logits are O(1) here).
The dst-side logit term t is broadcast per window via a K=1 ones-matmul of
a staged t-row, then reduced per edge with onehot*T_mat + ACT accumulate.
Leaky-dot products use att_l sign-folding into the weights + ACT Lrelu with
accum_out.  ELU is exp(min(x,0))+max(x,0)-1 with the -1 folded into the GRU
input bias.
"""

import numpy as np

from concourse import bacc, bass, mybir, tile
from concourse import bass_utils
from concourse.masks import make_identity

R = 8            # cores
P = 128
H = 128
NODE_IN = 44
EDGE_IN = 12
OUT_DIM = 512
MLP_H = 256
NUM_ATOM_EXTRA = 2
NUM_TIMESTEPS = 3

F32 = mybir.dt.float32
I32 = mybir.dt.int32
AF = mybir.ActivationFunctionType
OP = mybir.AluOpType


# ----------------------------------------------------------------------------
# host-side preprocessing
# ----------------------------------------------------------------------------

def prep(inputs):
    x = np.asarray(inputs["x"], np.float32)
    edge_attr = np.asarray(inputs["edge_attr"], np.float32)
    edge_index = np.asarray(inputs["edge_index"])
    batch = np.asarray(inputs["batch"])
    N = x.shape[0]
    M = int(batch.max()) + 1
    MPR = (M + R - 1) // R                      # mols per core (real)
    src, dst = edge_index[0].astype(np.int64), edge_index[1].astype(np.int64)

    ns = np.array([int(np.searchsorted(batch, c * MPR)) for c in range(R)] + [N])
    counts = np.diff(ns)
    NP = int(np.ceil(counts.max() / P) * P)
    W = NP // P
    MP = int(np.ceil(MPR / P) * P)
    MW = MP // P

    owner = np.searchsorted(ns[1:], dst, side="right")
    shard_edges = []
    maxT = 0
    for c in range(R):
        sel = np.where(owner == c)[0]
        d_loc = dst[sel] - ns[c]
        order = np.argsort(d_loc, kind="stable")
        sel, d_loc = sel[order], d_loc[order]
        win = d_loc // P
        cnt = np.bincount(win, minlength=W)
        maxT = max(maxT, int(np.ceil(cnt.max() / P)))
        shard_edges.append((sel, d_loc, cnt))
    T = maxT
    EW = T * P
    ES = W * EW
    EC = ES // P

    TNt = 0
    mol_streams = []
    for c in range(R):
        bl = batch[ns[c]:ns[c + 1]] - c * MPR
        mwin = bl // P
        cnt = np.bincount(mwin, minlength=MW)
        TNt = max(TNt, int(np.ceil(cnt.max() / P)))
        mol_streams.append((bl, mwin, cnt))
    MSC = MW * TNt
    MS = MSC * P

    def to_pc(a, cols):
        return np.ascontiguousarray(a.reshape(cols, P).T)

    cores = []
    aux = []
    for c in range(R):
        sel, d_loc, cnt = shard_edges[c]
        src_gid = np.zeros(ES, np.int64)
        tgate = np.zeros(ES, np.int64)
        satom = np.zeros(ES, np.int64)
        tatom = np.zeros(ES, np.int64)
        dcode = np.full(ES, 255.0, np.float32)
        ea_s = np.zeros((EDGE_IN, ES), np.float32)
        pos = 0
        for w in range(W):
            k = int(cnt[w])
            sl = slice(pos, pos + k)
            out = slice(w * EW, w * EW + k)
            e_ids = sel[sl]
            so = np.searchsorted(ns[1:], src[e_ids], side="right")
            s_loc = src[e_ids] - ns[so]
            src_gid[out] = so * NP + s_loc
            dl = d_loc[sl]
            dp, dw = dl % P, dl // P
            tgate[out] = (c * P + dp) * W + dw
            sp, sw = s_loc % P, s_loc // P
            satom[out] = (so * P + sp) * (2 * W) + sw
            tatom[out] = (c * P + dp) * (2 * W) + W + dw
            dcode[out] = (dl - w * P).astype(np.float32)
            ea_s[:, out] = edge_attr[e_ids].T
            pos += k

        bl, mwin, cnt_m = mol_streams[c]
        nc_ = counts[c]
        mol_nidx = np.zeros(MS, np.int64)
        mol_sidx = np.zeros(MS, np.int64)
        mol_tidx = np.zeros(MS, np.int64)
        mcode = np.full(MS, 255.0, np.float32)
        order = np.argsort(mwin, kind="stable")
        pos = 0
        for w in range(MW):
            k = int(cnt_m[w])
            ids = order[pos:pos + k]
            out = slice(w * TNt * P, w * TNt * P + k)
            mol_nidx[out] = ids
            vp, vw = ids % P, ids // P
            mol_sidx[out] = vp * W + vw
            m = bl[ids]
            mol_tidx[out] = (m % P) * MW + m // P
            mcode[out] = (m - w * P).astype(np.float32)
            pos += k

        xT = np.zeros((NODE_IN, NP), np.float32)
        xT[:, :nc_] = x[ns[c]:ns[c + 1]].T

        cores.append(dict(
            xT=xT, eaT=np.ascontiguousarray(ea_s),
            src_gid=to_pc(src_gid, EC).astype(np.int32),
            dcode=to_pc(dcode, EC).astype(np.float32),
            mol_nidx=to_pc(mol_nidx, MSC).astype(np.int32),
            mcode=to_pc(mcode, MSC).astype(np.float32),
        ))
        aux.append(dict(
            tgate=to_pc(tgate, EC).astype(np.int32),
            satom=to_pc(satom, EC).astype(np.int32),
            tatom=to_pc(tatom, EC).astype(np.int32),
            mol_sidx=to_pc(mol_sidx, MSC).astype(np.int32),
            mol_tidx=to_pc(mol_tidx, MSC).astype(np.int32),
        ))

    # ---------------- weight prep (shared across cores) ----------------
    g = {k: np.asarray(v, np.float32) for k, v in inputs.items()
         if k not in ("x", "edge_attr", "edge_index", "batch", "return_lats")}

    att_l = g["gate_att_l"]
    pos_idx = np.where(att_l >= 0)[0]
    neg_idx = np.where(att_l < 0)[0]
    perm = np.concatenate([pos_idx, neg_idx])
    kpos = int(len(pos_idx))
    scale = np.abs(att_l)[perm]
    W1 = g["gate_lin1_w"]
    w1x_f = W1[perm, :H] * scale[:, None]
    w1e_f = W1[perm, H:] * scale[:, None]

    cols = []
    colmap = {}

    def add(name, arr):
        arr = np.asarray(arr, np.float32)
        if arr.ndim == 1:
            arr = arr[:, None]
        assert arr.shape[0] <= P
        a = np.zeros((P, arr.shape[1]), np.float32)
        a[:arr.shape[0]] = arr
        start = sum(c[1].shape[1] for c in cols)
        cols.append((name, a))
        colmap[name] = (start, arr.shape[1])

    def gru_cols(pref, wih, whh, bih, bhh):
        bih_adj = bih - wih.sum(1)
        for i, gname in enumerate(("r", "z", "n")):
            add(f"{pref}_wih_{gname}", wih[i * H:(i + 1) * H].T)
            add(f"{pref}_whh_{gname}", whh[i * H:(i + 1) * H].T)
        add(f"{pref}_b_r", bih_adj[0:H] + bhh[0:H])
        add(f"{pref}_b_z", bih_adj[H:2 * H] + bhh[H:2 * H])
        add(f"{pref}_bhh_n", bhh[2 * H:])
        add(f"{pref}_bih_n", bih_adj[2 * H:])

    add("w1x_fT", w1x_f.T)
    add("gate_lin2T", g["gate_lin2_w"].T)
    add("lin1_b", g["lin1_b"])
    add("gate_att_r", g["gate_att_r"])
    add("gate_bias", g["gate_bias"])
    gru_cols("gru0", g["gru0_wih"], g["gru0_whh"], g["gru0_bih"], g["gru0_bhh"])
    for l in range(NUM_ATOM_EXTRA):
        add(f"atom{l}_linT", g["atom_lin_w"][l].T)
        add(f"atom{l}_att", np.stack([g["atom_att_src"][l], g["atom_att_dst"][l]], 1))
        add(f"atom{l}_bias", g["atom_bias"][l])
        gru_cols(f"atom{l}", g["atom_gru_wih"][l], g["atom_gru_whh"][l],
                 g["atom_gru_bih"][l], g["atom_gru_bhh"][l])
    add("mol_linT", g["mol_lin_w"].T)
    add("mol_att_src", g["mol_att_src"])
    add("mol_att_dst", g["mol_att_dst"])
    add("mol_bias", g["mol_bias"])
    gru_cols("mol", g["mol_gru_wih"], g["mol_gru_whh"], g["mol_gru_bih"],
             g["mol_gru_bhh"])
    for j in range(OUT_DIM // P):
        add(f"lin2T_{j}", g["lin2_w"][j * P:(j + 1) * P].T)
    add("lin2_b", g["lin2_b"].reshape(OUT_DIM // P, P).T)
    for j2 in range(MLP_H // P):
        for kc in range(OUT_DIM // P):
            add(f"mlp1T_{j2}_{kc}",
                g["mlp1_w"][j2 * P:(j2 + 1) * P, kc * P:(kc + 1) * P].T)
    add("mlp1_b", g["mlp1_b"].reshape(MLP_H // P, P).T)
    for kc in range(MLP_H // P):
        add(f"mlp2T_{kc}", g["mlp2_w"][:, kc * P:(kc + 1) * P].T)
    wpack = np.concatenate([c[1] for c in cols], axis=1)

    lin1_wT = np.ascontiguousarray(g["lin1_w"].T)
    w1e_fT = np.ascontiguousarray(w1e_f.T)
    mlp2_b = float(g["mlp2_b"].reshape(-1)[0])

    dims = dict(N=N, M=M, MPR=MPR, NP=NP, W=W, T=T, EW=EW, ES=ES, EC=EC,
                MP=MP, MW=MW, TNt=TNt, MS=MS, MSC=MSC, kpos=kpos,
                PW=wpack.shape[1], mlp2_b=mlp2_b, colmap=colmap, ns=ns)

    in_maps = []
    for c in range(R):
        m = dict(cores[c])
        m["wpack"] = wpack
        m["lin1_wT"] = lin1_wT
        m["w1e_fT"] = w1e_fT
        in_maps.append(m)
    return dims, in_maps, aux



# ----------------------------------------------------------------------------
# bass builder
# ----------------------------------------------------------------------------

def build(dims, debug_taps=False, phases=99):
    NP, W, T, EW, ES, EC = (dims[k] for k in ("NP", "W", "T", "EW", "ES", "EC"))
    MP, MW, TNt, MSC = (dims[k] for k in ("MP", "MW", "TNt", "MSC"))
    MS = dims["MS"]
    kpos = dims["kpos"]
    PW = dims["PW"]
    colmap = dims["colmap"]
    HP1 = H + 1
    CA = 132              # atom payload row: [hs(128) | s | pad3]
    CM = 264              # mol payload row: [xcur(128) | hs(128) | s | pad7]
    assert 0 < kpos < P, f"degenerate att_l sign split: kpos={kpos}"

    nc = bacc.Bacc("TRN2", target_bir_lowering=False, debug=False, num_devices=R)

    xT_d = nc.dram_tensor("xT", [NODE_IN, NP], F32, kind="ExternalInput")
    eaT_d = nc.dram_tensor("eaT", [EDGE_IN, ES], F32, kind="ExternalInput")
    srcg_d = nc.dram_tensor("src_gid", [P, EC], I32, kind="ExternalInput")
    dcode_d = nc.dram_tensor("dcode", [P, EC], F32, kind="ExternalInput")
    mnidx_d = nc.dram_tensor("mol_nidx", [P, MSC], I32, kind="ExternalInput")
    mcode_d = nc.dram_tensor("mcode", [P, MSC], F32, kind="ExternalInput")
    wpack_d = nc.dram_tensor("wpack", [P, PW], F32, kind="ExternalInput")
    lin1wT_d = nc.dram_tensor("lin1_wT", [NODE_IN, P], F32, kind="ExternalInput")
    w1efT_d = nc.dram_tensor("w1e_fT", [EDGE_IN, P], F32, kind="ExternalInput")
    out_d = nc.dram_tensor("out", [MP], F32, kind="ExternalOutput")

    def dbgt(name, shape):
        if debug_taps:
            return nc.dram_tensor(name, shape, F32, kind="ExternalOutput")
        return None

    dbg_h0 = dbgt("dbg_h0T", [P, NP])
    dbg_w0 = dbgt("dbg_w0", [P, EC])
    dbg_m0 = dbgt("dbg_m0T", [P, NP])
    dbg_x0 = dbgt("dbg_x0T", [P, NP])
    dbg_x2 = dbgt("dbg_x2T", [P, NP])
    dbg_ro = dbgt("dbg_roT", [P, MP])

    with tile.TileContext(nc) as tc:
        with tc.tile_pool(name="res", bufs=1) as res, \
             tc.tile_pool(name="big", bufs=1) as big, \
             tc.tile_pool(name="stp", bufs=1) as stp, \
             tc.tile_pool(name="sc", bufs=2) as sc, \
             tc.tile_pool(name="wk", bufs=2) as wk, \
             tc.tile_pool(name="gp", bufs=3) as gp, \
             tc.tile_pool(name="dram", bufs=1, space="DRAM") as dram, \
             tc.tile_pool(name="ps2", bufs=2, space="PSUM") as ps2, \
             tc.tile_pool(name="pscat", bufs=2, space="PSUM") as pscat, \
             tc.tile_pool(name="ptp", bufs=2, space="PSUM") as ptp, \
             tc.tile_pool(name="psml", bufs=1, space="PSUM") as psml:

            # ---------------- resident constants ----------------
            ident = res.tile([P, P], F32)
            make_identity(nc, ident[:])
            ones_row = res.tile([1, P], F32)
            nc.gpsimd.memset(ones_row[:], 1.0)
            iota_i = wk.tile([P, T * P], I32, tag="iotai")
            nc.gpsimd.iota(iota_i[:], pattern=[[0, T], [1, P]], base=0,
                           channel_multiplier=0)
            iota_e = res.tile([P, T * P], F32)
            nc.vector.tensor_copy(iota_e[:], iota_i[:])

            wp = res.tile([P, PW], F32)
            nc.sync.dma_start(wp[:], wpack_d[:])
            lin1_wT = res.tile([NODE_IN, P], F32)
            nc.sync.dma_start(lin1_wT[:], lin1wT_d[:])
            w1e_fT = res.tile([EDGE_IN, P], F32)
            nc.sync.dma_start(w1e_fT[:], w1efT_d[:])

            def wcol(name):
                s, n = colmap[name]
                return wp[:, s:s + n]

            srcg = res.tile([P, EC], I32)
            nc.sync.dma_start(srcg[:], srcg_d[:])
            dcf = res.tile([P, EC], F32)
            nc.sync.dma_start(dcf[:], dcode_d[:])

            # ---------------- DRAM buffers ----------------
            def dtile(shape, tg, shared=False):
                return dram.tile(shape, F32, tag=tg, name=tg,
                                 addr_space="Shared" if shared else "Local")

            h0T_dram = dtile([P, NP], "h0T")
            mT_dram = dtile([P, NP], "mT")
            xcA = dtile([P, NP], "xcA")
            xcB = dtile([P, NP], "xcB")
            tab0l = dtile([NP, 2 * H], "tab0l")
            tab0 = dtile([R * NP, 2 * H], "tab0", shared=True)
            tabA_l = [dtile([NP, CA], f"tabA_l{i}") for i in range(NUM_ATOM_EXTRA)]
            tabA = [dtile([R * NP, CA], f"tabA{i}", shared=True)
                    for i in range(NUM_ATOM_EXTRA)]
            mrows = dtile([NP, CM], "mrows")
            mstream = dtile([MS, CA], "mstream")
            tn_dram = dtile([NP, 1], "tn_dram")

            chunks = [(cs, min(512, NP - cs)) for cs in range(0, NP, 512)]
            mol_chunks = [(cs, min(512, MP - cs)) for cs in range(0, MP, 512)]

            def rows_ap(tab, cs, L):
                return tab[cs:cs + L, :].rearrange("(t p) h -> p t h", p=P)

            def s512(tag):
                return sc.tile([P, 512], F32, tag=tag, name=tag)

            rg = [list(range(R))]

            # =========================================================
            # GATE node phase -> tab0 rows ([p1 | g2]) + t0 staging
            # =========================================================
            t0stag = stp.tile([P, W], F32, tag="t0stag")
            for cs, L in chunks:
                nt = L // P
                xin = sc.tile([NODE_IN, 512], F32, tag="xin")
                nc.sync.dma_start(xin[:, :L], xT_d[:, cs:cs + L])
                pm = ps2.tile([P, 512], F32, tag="mm512")
                nc.tensor.matmul(out=pm[:, :L], lhsT=lin1_wT[:], rhs=xin[:, :L],
                                 start=True, stop=True)
                h0sb = s512("t1")
                nc.scalar.activation(h0sb[:, :L], pm[:, :L], AF.Lrelu,
                                     bias=wcol("lin1_b"), alpha=0.01)
                nc.sync.dma_start(h0T_dram[:, cs:cs + L], h0sb[:, :L])
                st = wk.tile([P, 4 * 2 * H], F32, tag="rstag0", name="rstag0")
                for nm, wname, off in (("p1", "w1x_fT", 0), ("g2", "gate_lin2T", H)):
                    pm2 = ps2.tile([P, 512], F32, tag="mm512")
                    nc.tensor.matmul(out=pm2[:, :L], lhsT=wcol(wname),
                                     rhs=h0sb[:, :L], start=True, stop=True)
                    psb = s512("t2")
                    nc.scalar.activation(psb[:, :L], pm2[:, :L], AF.Copy)
                    for t in range(nt):
                        tp = ptp.tile([P, P], F32, tag="tp")
                        nc.tensor.transpose(tp[:], psb[:, t * P:(t + 1) * P], ident[:])
                        nc.vector.tensor_copy(
                            st[:, t * 2 * H + off:t * 2 * H + off + H], tp[:])
                for t in range(nt):
                    w = cs // P + t
                    pt = psml.tile([P, 2], F32, tag="small")
                    nc.tensor.matmul(out=pt[:, 0:1], lhsT=h0sb[:, t * P:(t + 1) * P],
                                     rhs=wcol("gate_att_r"), start=True, stop=True)
                    nc.vector.tensor_copy(t0stag[:, w:w + 1], pt[:, 0:1])
                nc.sync.dma_start(rows_ap(tab0l, cs, L),
                                  st[:, :nt * 2 * H].rearrange(
                                      "p (t h) -> p t h", h=2 * H))
            if dbg_h0 is not None:
                for cs, L in chunks:
                    tmp = s512("t3")
                    nc.sync.dma_start(tmp[:, :L], h0T_dram[:, cs:cs + L])
                    nc.sync.dma_start(dbg_h0[:, cs:cs + L], tmp[:, :L])

            if phases >= 2:
                nc.gpsimd.collective_compute(
                    "AllGather", OP.bypass, replica_groups=rg,
                    ins=[tab0l.opt()], outs=[tab0.opt()])

            # =========================================================
            # shared helpers
            # =========================================================
            def t_rows_for(tstag_t):
                """Stage per-node t values to DRAM in node order; windows are
                then loaded back as [1, 128] rows."""
                nc.sync.dma_start(
                    tn_dram[:].rearrange("(w p) one -> p (w one)", p=P),
                    tstag_t[:])
                return tn_dram

            def tmat_for(tnd, w):
                """T_mat[p, nw] = t(window-w node nw) for every p — K=1
                broadcast matmul from a [1, 128] t row; returned in SBUF."""
                trow = wk.tile([1, P], F32, tag="trow", name="trow")
                nc.sync.dma_start(trow[:], tnd[w * P:(w + 1) * P, :].rearrange(
                    "v one -> one v"))
                tm = ptp.tile([P, P], F32, tag="tmat", name="tmat", bufs=1)
                nc.tensor.matmul(out=tm[:], lhsT=ones_row[:], rhs=trow[:],
                                 start=True, stop=True)
                tmsb = wk.tile([P, P], F32, tag="tmsb", name="tmsb")
                nc.vector.tensor_copy(tmsb[:], tm[:])
                return tmsb

            def onehot_for(codes, c0, tcount, iota):
                onehot = wk.tile([P, T * P], F32, tag="onehot", name="onehot")
                nc.vector.tensor_tensor(
                    out=onehot[:, :tcount * P].rearrange("p (t n) -> p t n",
                                                         t=tcount),
                    in0=codes[:, c0:c0 + tcount].to_broadcast([P, tcount, P]),
                    in1=iota[:, :tcount * P].rearrange("p (t n) -> p t n",
                                                       t=tcount),
                    op=OP.is_equal)
                return onehot

            def te_accum(onehot, g, tm, te_col):
                scr2 = wk.tile([P, P], F32, tag="scr2", name="scr2")
                nc.vector.tensor_tensor(out=scr2[:],
                                        in0=onehot[:, g * P:(g + 1) * P],
                                        in1=tm[:], op=OP.mult)
                nc.scalar.activation(scr2[:], scr2[:], AF.Identity,
                                     accum_out=te_col)

            def weight_and_scatter(gwin, stride, moff, wx, ws, onehot, tcount):
                rhs = wk.tile([P, T * HP1], F32, tag="rhs")
                r3 = rhs[:, :tcount * HP1].rearrange("p (t c) -> p t c", t=tcount)
                nc.vector.tensor_tensor(
                    out=r3[:, :, 0:H],
                    in0=gwin[:, :tcount * stride].rearrange(
                        "p (t c) -> p t c", t=tcount)[:, :, moff:moff + H],
                    in1=wx[:, ws].to_broadcast([P, tcount, H]), op=OP.mult)
                nc.vector.tensor_copy(r3[:, :, H:HP1],
                                      wx[:, ws].to_broadcast([P, tcount, 1]))
                return rhs

            def scat_epilogue(psum):
                den = wk.tile([P, 1], F32, tag="den")
                nc.vector.tensor_scalar_add(den[:], psum[:, H:HP1], 1e-16)
                rec = wk.tile([P, 1], F32, tag="rec")
                nc.vector.reciprocal(rec[:], den[:])
                msc = wk.tile([P, H], F32, tag="msc")
                nc.scalar.activation(msc[:], psum[:, 0:H], AF.Copy, scale=rec[:])
                tp = ptp.tile([P, P], F32, tag="tp", name="tp_e")
                nc.tensor.transpose(tp[:], msc[:], ident[:])
                return tp

            # =========================================================
            # GATE edge phase (single pass) -> mT_dram
            # =========================================================
            lp = stp.tile([P, EC], F32, tag="e1", name="lp")
            ln = stp.tile([P, EC], F32, tag="e2", name="ln")
            te = stp.tile([P, EC], F32, tag="e0", name="te")
            wx0 = stp.tile([P, EC], F32, tag="e3", name="wx0")
            tra0 = t_rows_for(t0stag)
            for w in range(W if phases >= 3 else 0):
                gwin = gp.tile([P, T * 2 * H], F32, tag="gath", name="gath")
                for g in range(T):
                    nc.gpsimd.indirect_dma_start(
                        out=gwin[:, g * 2 * H:(g + 1) * 2 * H], out_offset=None,
                        in_=tab0[:],
                        in_offset=bass.IndirectOffsetOnAxis(
                            ap=srcg[:, w * T + g:w * T + g + 1], axis=0))
                ea_w = gp.tile([EDGE_IN, EW], F32, tag="ea_w")
                nc.sync.dma_start(ea_w[:], eaT_d[:, w * EW:(w + 1) * EW])
                onehot = onehot_for(dcf, w * T, T, iota_e)
                tm = tmat_for(tra0, w)
                for g in range(T):
                    col = w * T + g
                    pq = ptp.tile([P, P], F32, tag="tp", name="tp_q")
                    nc.tensor.matmul(out=pq[:], lhsT=ea_w[:, g * P:(g + 1) * P],
                                     rhs=w1e_fT[:], start=True, stop=True)
                    xj = wk.tile([P, P], F32, tag="xj")
                    nc.vector.tensor_add(xj[:], pq[:],
                                         gwin[:, g * 2 * H:g * 2 * H + H])
                    scr = wk.tile([P, P], F32, tag="scr")
                    nc.scalar.activation(scr[:, :kpos], xj[:, :kpos], AF.Lrelu,
                                         alpha=0.01, accum_out=lp[:, col:col + 1])
                    nc.scalar.activation(scr[:, kpos:], xj[:, kpos:], AF.Lrelu,
                                         alpha=0.01, accum_out=ln[:, col:col + 1])
                    te_accum(onehot, g, tm, te[:, col:col + 1])
                ws = slice(w * T, (w + 1) * T)
                nc.vector.tensor_tensor(out=wx0[:, ws], in0=lp[:, ws],
                                        in1=ln[:, ws], op=OP.subtract)
                nc.vector.tensor_tensor(out=wx0[:, ws], in0=wx0[:, ws],
                                        in1=te[:, ws], op=OP.add)
                nc.scalar.activation(wx0[:, ws], wx0[:, ws], AF.Lrelu, alpha=0.01)
                nc.scalar.activation(wx0[:, ws], wx0[:, ws], AF.Exp)
                rhs = weight_and_scatter(gwin, 2 * H, H, wx0, ws, onehot, T)
                psum = pscat.tile([P, HP1], F32, tag="scat")
                for g in range(T):
                    nc.tensor.matmul(out=psum[:],
                                     lhsT=onehot[:, g * P:(g + 1) * P],
                                     rhs=rhs[:, g * HP1:(g + 1) * HP1],
                                     start=(g == 0), stop=(g == T - 1))
                tp = scat_epilogue(psum)
                mloc = wk.tile([P, P], F32, tag="mloc")
                nc.vector.tensor_copy(mloc[:], tp[:])
                nc.sync.dma_start(mT_dram[:, w * P:(w + 1) * P], mloc[:])
            if dbg_w0 is not None and phases >= 3:
                nc.sync.dma_start(dbg_w0[:], wx0[:])
            if dbg_m0 is not None and phases >= 3:
                for cs, L in chunks:
                    tmp = s512("t3")
                    nc.sync.dma_start(tmp[:, :L], mT_dram[:, cs:cs + L])
                    nc.sync.dma_start(dbg_m0[:, cs:cs + L], tmp[:, :L])

            # =========================================================
            # GRU (chunked over columns)
            # =========================================================
            def gru_chunk(pref, bias_name, m_sb, h_t, L, emit):
                bias = wcol(bias_name)
                t1 = s512("t1")
                nc.gpsimd.tensor_scalar(out=t1[:, :L], in0=m_sb[:, :L],
                                        scalar1=bias, scalar2=0.0,
                                        op0=OP.add, op1=OP.min)
                t2 = s512("t2")
                nc.gpsimd.tensor_scalar(out=t2[:, :L], in0=m_sb[:, :L],
                                        scalar1=bias, scalar2=0.0,
                                        op0=OP.add, op1=OP.max)
                t3 = s512("t3")
                nc.scalar.activation(t3[:, :L], t1[:, :L], AF.Exp)
                q = s512("q")
                nc.vector.tensor_add(q[:, :L], t3[:, :L], t2[:, :L])
                gate_sb = {}
                for gname in ("r", "z"):
                    pg = ps2.tile([P, 512], F32, tag="mm512")
                    nc.tensor.matmul(out=pg[:, :L],
                                     lhsT=wcol(f"{pref}_wih_{gname}"),
                                     rhs=q[:, :L], start=True, stop=False)
                    nc.tensor.matmul(out=pg[:, :L],
                                     lhsT=wcol(f"{pref}_whh_{gname}"),
                                     rhs=h_t[:, :L], start=False, stop=True)
                    gsb = s512(f"g{gname}")
                    nc.scalar.activation(gsb[:, :L], pg[:, :L], AF.Sigmoid,
                                         bias=wcol(f"{pref}_b_{gname}"))
                    gate_sb[gname] = gsb
                pn1 = ps2.tile([P, 512], F32, tag="mm512")
                nc.tensor.matmul(out=pn1[:, :L], lhsT=wcol(f"{pref}_wih_n"),
                                 rhs=q[:, :L], start=True, stop=True)
                pn2 = ps2.tile([P, 512], F32, tag="mm512")
                nc.tensor.matmul(out=pn2[:, :L], lhsT=wcol(f"{pref}_whh_n"),
                                 rhs=h_t[:, :L], start=True, stop=True)
                hnb = s512("hnb")
                nc.scalar.activation(hnb[:, :L], pn2[:, :L], AF.Identity,
                                     bias=wcol(f"{pref}_bhh_n"))
                nc.vector.tensor_tensor(out=hnb[:, :L], in0=gate_sb["r"][:, :L],
                                        in1=hnb[:, :L], op=OP.mult)
                nc.vector.tensor_tensor(out=hnb[:, :L], in0=pn1[:, :L],
                                        in1=hnb[:, :L], op=OP.add)
                n_sb = s512("n_sb")
                nc.scalar.activation(n_sb[:, :L], hnb[:, :L], AF.Tanh,
                                     bias=wcol(f"{pref}_bih_n"))
                d_sb = s512("d_sb")
                nc.vector.tensor_tensor(out=d_sb[:, :L], in0=h_t[:, :L],
                                        in1=n_sb[:, :L], op=OP.subtract)
                nc.vector.tensor_tensor(out=d_sb[:, :L], in0=gate_sb["z"][:, :L],
                                        in1=d_sb[:, :L], op=OP.mult)
                nc.vector.tensor_tensor(out=d_sb[:, :L], in0=n_sb[:, :L],
                                        in1=d_sb[:, :L], op=OP.add)
                xo = s512("xo")
                nc.scalar.activation(xo[:, :L], d_sb[:, :L], AF.Relu)
                emit(xo)

            def gru_phase(pref, bias_name, h_dram, xout_dram):
                for cs, L in chunks:
                    m_sb = s512("m_sb")
                    nc.sync.dma_start(m_sb[:, :L], mT_dram[:, cs:cs + L])
                    h_t = s512("h_t")
                    nc.sync.dma_start(h_t[:, :L], h_dram[:, cs:cs + L])
                    gru_chunk(pref, bias_name, m_sb, h_t, L,
                              lambda xo, cs=cs, L=L: nc.sync.dma_start(
                                  xout_dram[:, cs:cs + L], xo[:, :L]))

            if phases >= 4:
                gru_phase("gru0", "gate_bias", h0T_dram, xcA)
            if dbg_x0 is not None and phases >= 4:
                for cs, L in chunks:
                    tmp = s512("t3")
                    nc.sync.dma_start(tmp[:, :L], xcA[:, cs:cs + L])
                    nc.sync.dma_start(dbg_x0[:, cs:cs + L], tmp[:, :L])

            # =========================================================
            # Atom layers
            # =========================================================
            xin_t, xout_t = xcA, xcB
            for l in range(NUM_ATOM_EXTRA if phases >= 5 else 0):
                tstag_a = stp.tile([P, W], F32, tag="t0stag", name="tstag_a")
                for cs, L in chunks:
                    nt = L // P
                    xc_sb = s512("m_sb")
                    nc.sync.dma_start(xc_sb[:, :L], xin_t[:, cs:cs + L])
                    pm = ps2.tile([P, 512], F32, tag="mm512")
                    nc.tensor.matmul(out=pm[:, :L], lhsT=wcol(f"atom{l}_linT"),
                                     rhs=xc_sb[:, :L], start=True, stop=True)
                    hs_sb = s512("t1")
                    nc.scalar.activation(hs_sb[:, :L], pm[:, :L], AF.Copy)
                    st = wk.tile([P, 4 * CA], F32, tag="rstagA", name="rstagA")
                    for t in range(nt):
                        w = cs // P + t
                        tp = ptp.tile([P, P], F32, tag="tp")
                        nc.tensor.transpose(tp[:], hs_sb[:, t * P:(t + 1) * P],
                                            ident[:])
                        nc.vector.tensor_copy(st[:, t * CA:t * CA + H], tp[:])
                        pt = psml.tile([P, 2], F32, tag="small")
                        nc.tensor.matmul(out=pt[:],
                                         lhsT=hs_sb[:, t * P:(t + 1) * P],
                                         rhs=wcol(f"atom{l}_att"), start=True,
                                         stop=True)
                        nc.vector.tensor_copy(st[:, t * CA + H:t * CA + H + 1],
                                              pt[:, 0:1])
                        nc.vector.tensor_copy(tstag_a[:, w:w + 1], pt[:, 1:2])
                    nc.sync.dma_start(rows_ap(tabA_l[l], cs, L),
                                      st[:, :nt * CA].rearrange(
                                          "p (t h) -> p t h", h=CA))
                nc.gpsimd.collective_compute(
                    "AllGather", OP.bypass, replica_groups=rg,
                    ins=[tabA_l[l].opt()], outs=[tabA[l].opt()])

                wxa = stp.tile([P, EC], F32, tag="e3", name="wxa")
                tra_a = t_rows_for(tstag_a)
                for w in range(W):
                    gwin = gp.tile([P, T * CA], F32, tag="gath", name="gath")
                    for g in range(T):
                        nc.gpsimd.indirect_dma_start(
                            out=gwin[:, g * CA:(g + 1) * CA], out_offset=None,
                            in_=tabA[l][:],
                            in_offset=bass.IndirectOffsetOnAxis(
                                ap=srcg[:, w * T + g:w * T + g + 1], axis=0))
                    onehot = onehot_for(dcf, w * T, T, iota_e)
                    tm = tmat_for(tra_a, w)
                    for g in range(T):
                        col = w * T + g
                        te_accum(onehot, g, tm, te[:, col:col + 1])
                        # lp col <- s from payload
                        nc.vector.tensor_copy(
                            lp[:, col:col + 1],
                            gwin[:, g * CA + H:g * CA + H + 1])
                    ws = slice(w * T, (w + 1) * T)
                    nc.vector.tensor_tensor(out=wxa[:, ws], in0=lp[:, ws],
                                            in1=te[:, ws], op=OP.add)
                    nc.scalar.activation(wxa[:, ws], wxa[:, ws], AF.Lrelu,
                                         alpha=0.01)
                    nc.scalar.activation(wxa[:, ws], wxa[:, ws], AF.Exp)
                    rhs = weight_and_scatter(gwin, CA, 0, wxa, ws, onehot, T)
                    psum = pscat.tile([P, HP1], F32, tag="scat")
                    for g in range(T):
                        nc.tensor.matmul(out=psum[:],
                                         lhsT=onehot[:, g * P:(g + 1) * P],
                                         rhs=rhs[:, g * HP1:(g + 1) * HP1],
                                         start=(g == 0), stop=(g == T - 1))
                    tp = scat_epilogue(psum)
                    mloc = wk.tile([P, P], F32, tag="mloc")
                    nc.vector.tensor_copy(mloc[:], tp[:])
                    nc.sync.dma_start(mT_dram[:, w * P:(w + 1) * P], mloc[:])

                gru_phase(f"atom{l}", f"atom{l}_bias", xin_t, xout_t)
                xin_t, xout_t = xout_t, xin_t
            xcur_d = xin_t
            if dbg_x2 is not None and phases >= 5:
                for cs, L in chunks:
                    tmp = s512("t3")
                    nc.sync.dma_start(tmp[:, :L], xcur_d[:, cs:cs + L])
                    nc.sync.dma_start(dbg_x2[:, cs:cs + L], tmp[:, :L])

            # =========================================================
            # Mol phase
            # =========================================================
            outT = big.tile([P, MP], F32, tag="outT")
            if phases < 6:
                nc.gpsimd.memset(outT[:], 0.0)
            mni = res.tile([P, MSC], I32)
            nc.sync.dma_start(mni[:], mnidx_d[:])
            mcf = res.tile([P, MSC], F32)
            nc.sync.dma_start(mcf[:], mcode_d[:])

            ngr = (TNt + T - 1) // T
            for cs, L in (chunks if phases >= 6 else []):
                nt = L // P
                xc_sb = s512("m_sb")
                nc.sync.dma_start(xc_sb[:, :L], xcur_d[:, cs:cs + L])
                pm = ps2.tile([P, 512], F32, tag="mm512")
                nc.tensor.matmul(out=pm[:, :L], lhsT=wcol("mol_linT"),
                                 rhs=xc_sb[:, :L], start=True, stop=True)
                hs_sb = s512("t1")
                nc.scalar.activation(hs_sb[:, :L], pm[:, :L], AF.Copy)
                st = wk.tile([P, 4 * CM], F32, tag="rstagM", name="rstagM")
                for t in range(nt):
                    tp = ptp.tile([P, P], F32, tag="tp")
                    nc.tensor.transpose(tp[:], xc_sb[:, t * P:(t + 1) * P],
                                        ident[:])
                    nc.vector.tensor_copy(st[:, t * CM:t * CM + H], tp[:])
                    tp2 = ptp.tile([P, P], F32, tag="tp")
                    nc.tensor.transpose(tp2[:], hs_sb[:, t * P:(t + 1) * P],
                                        ident[:])
                    nc.vector.tensor_copy(st[:, t * CM + H:t * CM + 2 * H], tp2[:])
                    pt = psml.tile([P, 2], F32, tag="small")
                    nc.tensor.matmul(out=pt[:, 0:1],
                                     lhsT=hs_sb[:, t * P:(t + 1) * P],
                                     rhs=wcol("mol_att_src"), start=True,
                                     stop=True)
                    nc.vector.tensor_copy(
                        st[:, t * CM + 2 * H:t * CM + 2 * H + 1], pt[:, 0:1])
                nc.sync.dma_start(
                    mrows[cs:cs + L, :].rearrange("(t p) h -> p t h", p=P),
                    st[:, :nt * CM].rearrange("p (t h) -> p t h", h=CM))

            # readout: gather combined rows; scatter xcur part; stage [hs|s]
            for w in range(MW if phases >= 6 else 0):
                psum = pscat.tile([P, HP1], F32, tag="scat", name="mpsum")
                for gi in range(ngr):
                    g0 = gi * T
                    gt = min(T, TNt - g0)
                    c0 = w * TNt + g0
                    gath = gp.tile([P, T * CM], F32, tag="gath", name="gath")
                    for g in range(gt):
                        nc.gpsimd.indirect_dma_start(
                            out=gath[:, g * CM:(g + 1) * CM], out_offset=None,
                            in_=mrows[:],
                            in_offset=bass.IndirectOffsetOnAxis(
                                ap=mni[:, c0 + g:c0 + g + 1], axis=0))
                        nc.sync.dma_start(
                            mstream[(c0 + g) * P:(c0 + g + 1) * P, :],
                            gath[:, g * CM + H:g * CM + H + CA])
                    onehot = onehot_for(mcf, c0, gt, iota_e)
                    for g in range(gt):
                        nc.tensor.matmul(
                            out=psum[:, 0:H],
                            lhsT=onehot[:, g * P:(g + 1) * P],
                            rhs=gath[:, g * CM:g * CM + H],
                            start=(gi == 0 and g == 0),
                            stop=(gi == ngr - 1 and g == gt - 1))
                rsb = wk.tile([P, H], F32, tag="msc")
                nc.scalar.activation(rsb[:], psum[:, 0:H], AF.Relu)
                tp = ptp.tile([P, P], F32, tag="tp", name="tp_m")
                nc.tensor.transpose(tp[:], rsb[:], ident[:])
                nc.vector.tensor_copy(outT[:, w * P:(w + 1) * P], tp[:])
            if dbg_ro is not None:
                nc.sync.dma_start(dbg_ro[:], outT[:])

            mGT = big.tile([P, MP], F32, tag="mGT")
            mtstag = stp.tile([P, W], F32, tag="t0stag", name="mtstag")
            wsm = stp.tile([P, MSC], F32, tag="wsm", name="wsm")
            tem = stp.tile([P, MSC], F32, tag="tem", name="tem")
            for ts in range(NUM_TIMESTEPS if phases >= 6 else 0):
                for cs, L in mol_chunks:
                    pm = ps2.tile([P, 512], F32, tag="mm512")
                    nc.tensor.matmul(out=pm[:, :L], lhsT=wcol("mol_linT"),
                                     rhs=outT[:, cs:cs + L], start=True,
                                     stop=True)
                    hd_sb = s512("t1")
                    nc.scalar.activation(hd_sb[:, :L], pm[:, :L], AF.Copy)
                    for t in range(L // P):
                        w = cs // P + t
                        pt = psml.tile([P, 2], F32, tag="small")
                        nc.tensor.matmul(out=pt[:, 0:1],
                                         lhsT=hd_sb[:, t * P:(t + 1) * P],
                                         rhs=wcol("mol_att_dst"), start=True,
                                         stop=True)
                        nc.vector.tensor_copy(mtstag[:, w:w + 1], pt[:, 0:1])
                tra_m = t_rows_for(mtstag)
                for w in range(MW):
                    tm = tmat_for(tra_m, w)
                    psum = pscat.tile([P, HP1], F32, tag="scat", name="mpsum")
                    for gi in range(ngr):
                        g0 = gi * T
                        gt = min(T, TNt - g0)
                        c0 = w * TNt + g0
                        ms = gp.tile([P, T * CM], F32, tag="gath", name="gath")
                        nc.sync.dma_start(
                            ms[:, :gt * CA],
                            mstream[c0 * P:(c0 + gt) * P, :].rearrange(
                                "(t p) h -> p t h", p=P))
                        onehot = onehot_for(mcf, c0, gt, iota_e)
                        for g in range(gt):
                            col = c0 + g
                            te_accum(onehot, g, tm, tem[:, col:col + 1])
                            nc.vector.tensor_tensor(
                                out=wsm[:, col:col + 1],
                                in0=ms[:, g * CA + H:g * CA + H + 1],
                                in1=tem[:, col:col + 1], op=OP.add)
                        wsl = slice(c0, c0 + gt)
                        nc.scalar.activation(wsm[:, wsl], wsm[:, wsl], AF.Lrelu,
                                             alpha=0.01)
                        nc.scalar.activation(wsm[:, wsl], wsm[:, wsl], AF.Exp)
                        rhs = weight_and_scatter(ms, CA, 0, wsm, wsl, onehot, gt)
                        for g in range(gt):
                            nc.tensor.matmul(
                                out=psum[:],
                                lhsT=onehot[:, g * P:(g + 1) * P],
                                rhs=rhs[:, g * HP1:(g + 1) * HP1],
                                start=(gi == 0 and g == 0),
                                stop=(gi == ngr - 1 and g == gt - 1))
                    tp = scat_epilogue(psum)
                    nc.vector.tensor_copy(mGT[:, w * P:(w + 1) * P], tp[:])

                for cs, L in mol_chunks:
                    m_sb = s512("m_sb")
                    nc.vector.tensor_copy(m_sb[:, :L], mGT[:, cs:cs + L])
                    h_t = s512("h_t")
                    nc.vector.tensor_copy(h_t[:, :L], outT[:, cs:cs + L])
                    gru_chunk("mol", "mol_bias", m_sb, h_t, L,
                              lambda xo, cs=cs, L=L: nc.vector.tensor_copy(
                                  outT[:, cs:cs + L], xo[:, :L]))

            # =========================================================
            # Final MLP
            # =========================================================
            h1 = [big.tile([P, MP], F32, tag=f"h1_{j}", name=f"h1_{j}")
                  for j in range(OUT_DIM // P)]
            for j in range(OUT_DIM // P):
                for cs, L in mol_chunks:
                    pm = ps2.tile([P, 512], F32, tag="mm512")
                    nc.tensor.matmul(out=pm[:, :L], lhsT=wcol(f"lin2T_{j}"),
                                     rhs=outT[:, cs:cs + L], start=True,
                                     stop=True)
                    nc.scalar.activation(h1[j][:, cs:cs + L], pm[:, :L], AF.Relu,
                                         bias=wcol("lin2_b")[:, j:j + 1])
            h2 = [big.tile([P, MP], F32, tag=f"h2_{j}", name=f"h2_{j}")
                  for j in range(MLP_H // P)]
            for j2 in range(MLP_H // P):
                for cs, L in mol_chunks:
                    pm = ps2.tile([P, 512], F32, tag="mm512")
                    for kc in range(OUT_DIM // P):
                        nc.tensor.matmul(out=pm[:, :L],
                                         lhsT=wcol(f"mlp1T_{j2}_{kc}"),
                                         rhs=h1[kc][:, cs:cs + L],
                                         start=(kc == 0),
                                         stop=(kc == OUT_DIM // P - 1))
                    nc.scalar.activation(h2[j2][:, cs:cs + L], pm[:, :L], AF.Relu,
                                         bias=wcol("mlp1_b")[:, j2:j2 + 1])
            osb = stp.tile([1, MP], F32, tag="osb")
            for cs, L in mol_chunks:
                pm = psml.tile([1, 512], F32, tag="small", name="finps")
                for kc in range(MLP_H // P):
                    nc.tensor.matmul(out=pm[:, :L], lhsT=wcol(f"mlp2T_{kc}"),
                                     rhs=h2[kc][:, cs:cs + L],
                                     start=(kc == 0), stop=(kc == MLP_H // P - 1))
                nc.vector.tensor_scalar_add(osb[:, cs:cs + L], pm[:, :L],
                                            dims["mlp2_b"])
            nc.sync.dma_start(out_d[:].rearrange("(one m) -> one m", one=1),
                              osb[:])

    nc.compile()
    return nc


# ----------------------------------------------------------------------------
# entry point
# ----------------------------------------------------------------------------

def run(inputs, debug_taps=False, trace=False):
    dims, in_maps, _aux = prep(inputs)
    nc = build(dims, debug_taps=debug_taps)
    res = bass_utils.run_bass_kernel_spmd(
        nc, in_maps, core_ids=list(range(R)), trace=trace)
    MPR = dims["MPR"]
    out = np.concatenate([res.results[c]["out"][:MPR].reshape(-1)
                          for c in range(R)])
    return out[:dims["M"]].astype(np.float32), res, dims


def kernel(**inputs):
    out, _, _ = run(inputs)
    return out



# revision 24
# speedup vs baseline: 1.2567x; 1.2567x over previous
"""AttentiveFP GNN (nn_AFP_jittable) as a distributed Bass kernel on 8 TRN2
NeuronCores.

Sharding: molecules are split across the 8 cores; nodes at molecule
boundaries (padded to NP); each edge is owned by the core owning its dst
node.  Edges are sorted by dst and grouped into 128-node windows with a
uniform tile budget T per window (SPMD-uniform shapes).  Per GAT layer:
node-phase matmuls run in transposed [k, n] layout, payload rows are
PE-transposed into a local DRAM table, AllGathered, then per-edge 512B-row
indirect-DMA gathers (one 128-row gather per tile) feed one-hot scatter
matmuls that accumulate [node_window, H+1] (messages + softmax denominator)
in PSUM.  Segment softmax skips max-subtraction (logits are O(1) here).
The dst-side logit term t is broadcast per window via a K=1 ones-matmul of
a staged t-row, then reduced per edge with onehot*T_mat + ACT accumulate.
Leaky-dot products use att_l sign-folding into the weights + ACT Lrelu with
accum_out.  ELU is exp(min(x,0))+max(x,0)-1 with the -1 folded into the GRU
input bias.
"""

import numpy as np

from concourse import bacc, bass, mybir, tile
from concourse import bass_utils
from concourse.masks import make_identity

R = 8            # cores
P = 128
H = 128
NODE_IN = 44
EDGE_IN = 12
OUT_DIM = 512
MLP_H = 256
NUM_ATOM_EXTRA = 2
NUM_TIMESTEPS = 3

F32 = mybir.dt.float32
I32 = mybir.dt.int32
AF = mybir.ActivationFunctionType
OP = mybir.AluOpType


# ----------------------------------------------------------------------------
# host-side preprocessing
# ----------------------------------------------------------------------------

def prep(inputs):
    x = np.asarray(inputs["x"], np.float32)
    edge_attr = np.asarray(inputs["edge_attr"], np.float32)
    edge_index = np.asarray(inputs["edge_index"])
    batch = np.asarray(inputs["batch"])
    N = x.shape[0]
    M = int(batch.max()) + 1
    MPR = (M + R - 1) // R                      # mols per core (real)
    src, dst = edge_index[0].astype(np.int64), edge_index[1].astype(np.int64)

    ns = np.array([int(np.searchsorted(batch, c * MPR)) for c in range(R)] + [N])
    counts = np.diff(ns)
    NP = int(np.ceil(counts.max() / P) * P)
    W = NP // P
    MP = int(np.ceil(MPR / P) * P)
    MW = MP // P

    owner = np.searchsorted(ns[1:], dst, side="right")
    shard_edges = []
    maxT = 0
    for c in range(R):
        sel = np.where(owner == c)[0]
        d_loc = dst[sel] - ns[c]
        order = np.argsort(d_loc, kind="stable")
        sel, d_loc = sel[order], d_loc[order]
        win = d_loc // P
        cnt = np.bincount(win, minlength=W)
        maxT = max(maxT, int(np.ceil(cnt.max() / P)))
        shard_edges.append((sel, d_loc, cnt))
    T = maxT
    EW = T * P
    ES = W * EW
    EC = ES // P

    # table quartering for staging/AllGather overlap: tables are stored
    # quarter-major ([Q, R, NPq, :]) so each quarter AllGathers into a
    # contiguous region as soon as its rows are staged
    Q = 1          # shared DRAM collective outputs allow a single writer
    NPq = NP // Q

    TNt = 0
    mol_streams = []
    for c in range(R):
        bl = batch[ns[c]:ns[c + 1]] - c * MPR
        mwin = bl // P
        cnt = np.bincount(mwin, minlength=MW)
        TNt = max(TNt, int(np.ceil(cnt.max() / P)))
        mol_streams.append((bl, mwin, cnt))
    MSC = MW * TNt
    MS = MSC * P

    def to_pc(a, cols):
        return np.ascontiguousarray(a.reshape(cols, P).T)

    cores = []
    aux = []
    for c in range(R):
        sel, d_loc, cnt = shard_edges[c]
        src_gid = np.zeros(ES, np.int64)
        tgate = np.zeros(ES, np.int64)
        satom = np.zeros(ES, np.int64)
        tatom = np.zeros(ES, np.int64)
        dcode = np.full(ES, 255.0, np.float32)
        ea_s = np.zeros((EDGE_IN, ES), np.float32)
        pos = 0
        for w in range(W):
            k = int(cnt[w])
            sl = slice(pos, pos + k)
            out = slice(w * EW, w * EW + k)
            e_ids = sel[sl]
            so = np.searchsorted(ns[1:], src[e_ids], side="right")
            s_loc = src[e_ids] - ns[so]
            src_gid[out] = (s_loc // NPq) * (R * NPq) + so * NPq + (s_loc % NPq)
            dl = d_loc[sl]
            dp, dw = dl % P, dl // P
            tgate[out] = (c * P + dp) * W + dw
            sp, sw = s_loc % P, s_loc // P
            satom[out] = (so * P + sp) * (2 * W) + sw
            tatom[out] = (c * P + dp) * (2 * W) + W + dw
            dcode[out] = (dl - w * P).astype(np.float32)
            ea_s[:, out] = edge_attr[e_ids].T
            pos += k

        bl, mwin, cnt_m = mol_streams[c]
        nc_ = counts[c]
        mol_nidx = np.zeros(MS, np.int64)
        mol_sidx = np.zeros(MS, np.int64)
        mol_tidx = np.zeros(MS, np.int64)
        mcode = np.full(MS, 255.0, np.float32)
        order = np.argsort(mwin, kind="stable")
        pos = 0
        for w in range(MW):
            k = int(cnt_m[w])
            ids = order[pos:pos + k]
            out = slice(w * TNt * P, w * TNt * P + k)
            mol_nidx[out] = ids
            vp, vw = ids % P, ids // P
            mol_sidx[out] = vp * W + vw
            m = bl[ids]
            mol_tidx[out] = (m % P) * MW + m // P
            mcode[out] = (m - w * P).astype(np.float32)
            pos += k

        xT = np.zeros((NODE_IN, NP), np.float32)
        xT[:, :nc_] = x[ns[c]:ns[c + 1]].T

        cores.append(dict(
            xT=xT, eaT=np.ascontiguousarray(ea_s),
            src_gid=to_pc(src_gid, EC).astype(np.int32),
            dcode=to_pc(dcode, EC).astype(np.float32),
            mol_nidx=to_pc(mol_nidx, MSC).astype(np.int32),
            mcode=to_pc(mcode, MSC).astype(np.float32),
        ))
        aux.append(dict(
            tgate=to_pc(tgate, EC).astype(np.int32),
            satom=to_pc(satom, EC).astype(np.int32),
            tatom=to_pc(tatom, EC).astype(np.int32),
            mol_sidx=to_pc(mol_sidx, MSC).astype(np.int32),
            mol_tidx=to_pc(mol_tidx, MSC).astype(np.int32),
        ))

    # ---------------- weight prep (shared across cores) ----------------
    g = {k: np.asarray(v, np.float32) for k, v in inputs.items()
         if k not in ("x", "edge_attr", "edge_index", "batch", "return_lats")}

    att_l = g["gate_att_l"]
    pos_idx = np.where(att_l >= 0)[0]
    neg_idx = np.where(att_l < 0)[0]
    perm = np.concatenate([pos_idx, neg_idx])
    kpos = int(len(pos_idx))
    scale = np.abs(att_l)[perm]
    W1 = g["gate_lin1_w"]
    w1x_f = W1[perm, :H] * scale[:, None]
    w1e_f = W1[perm, H:] * scale[:, None]

    cols = []
    colmap = {}

    def add(name, arr):
        arr = np.asarray(arr, np.float32)
        if arr.ndim == 1:
            arr = arr[:, None]
        assert arr.shape[0] <= P
        a = np.zeros((P, arr.shape[1]), np.float32)
        a[:arr.shape[0]] = arr
        start = sum(c[1].shape[1] for c in cols)
        cols.append((name, a))
        colmap[name] = (start, arr.shape[1])

    def gru_cols(pref, wih, whh, bih, bhh):
        bih_adj = bih - wih.sum(1)
        for i, gname in enumerate(("r", "z", "n")):
            add(f"{pref}_wih_{gname}", wih[i * H:(i + 1) * H].T)
            add(f"{pref}_whh_{gname}", whh[i * H:(i + 1) * H].T)
        add(f"{pref}_b_r", bih_adj[0:H] + bhh[0:H])
        add(f"{pref}_b_z", bih_adj[H:2 * H] + bhh[H:2 * H])
        add(f"{pref}_bhh_n", bhh[2 * H:])
        add(f"{pref}_bih_n", bih_adj[2 * H:])

    add("w1x_fT", w1x_f.T)
    add("gate_lin2T", g["gate_lin2_w"].T)
    add("lin1_b", g["lin1_b"])
    add("gate_att_r", g["gate_att_r"])
    add("gate_bias", g["gate_bias"])
    gru_cols("gru0", g["gru0_wih"], g["gru0_whh"], g["gru0_bih"], g["gru0_bhh"])
    for l in range(NUM_ATOM_EXTRA):
        add(f"atom{l}_linT", g["atom_lin_w"][l].T)
        add(f"atom{l}_att", np.stack([g["atom_att_src"][l], g["atom_att_dst"][l]], 1))
        add(f"atom{l}_bias", g["atom_bias"][l])
        gru_cols(f"atom{l}", g["atom_gru_wih"][l], g["atom_gru_whh"][l],
                 g["atom_gru_bih"][l], g["atom_gru_bhh"][l])
    add("mol_linT", g["mol_lin_w"].T)
    add("mol_att_src", g["mol_att_src"])
    add("mol_att_dst", g["mol_att_dst"])
    add("mol_bias", g["mol_bias"])
    gru_cols("mol", g["mol_gru_wih"], g["mol_gru_whh"], g["mol_gru_bih"],
             g["mol_gru_bhh"])
    for j in range(OUT_DIM // P):
        add(f"lin2T_{j}", g["lin2_w"][j * P:(j + 1) * P].T)
    add("lin2_b", g["lin2_b"].reshape(OUT_DIM // P, P).T)
    for j2 in range(MLP_H // P):
        for kc in range(OUT_DIM // P):
            add(f"mlp1T_{j2}_{kc}",
                g["mlp1_w"][j2 * P:(j2 + 1) * P, kc * P:(kc + 1) * P].T)
    add("mlp1_b", g["mlp1_b"].reshape(MLP_H // P, P).T)
    for kc in range(MLP_H // P):
        add(f"mlp2T_{kc}", g["mlp2_w"][:, kc * P:(kc + 1) * P].T)
    wpack = np.concatenate([c[1] for c in cols], axis=1)

    lin1_wT = np.ascontiguousarray(g["lin1_w"].T)
    w1e_fT = np.ascontiguousarray(w1e_f.T)
    mlp2_b = float(g["mlp2_b"].reshape(-1)[0])

    dims = dict(N=N, M=M, MPR=MPR, NP=NP, W=W, T=T, EW=EW, ES=ES, EC=EC,
                MP=MP, MW=MW, TNt=TNt, MS=MS, MSC=MSC, kpos=kpos, Q=Q,
                PW=wpack.shape[1], mlp2_b=mlp2_b, colmap=colmap, ns=ns)

    in_maps = []
    for c in range(R):
        m = dict(cores[c])
        m["wpack"] = wpack
        m["lin1_wT"] = lin1_wT
        m["w1e_fT"] = w1e_fT
        in_maps.append(m)
    return dims, in_maps, aux



# ----------------------------------------------------------------------------
# bass builder
# ----------------------------------------------------------------------------

def build(dims, debug_taps=False, phases=99):
    NP, W, T, EW, ES, EC = (dims[k] for k in ("NP", "W", "T", "EW", "ES", "EC"))
    MP, MW, TNt, MSC = (dims[k] for k in ("MP", "MW", "TNt", "MSC"))
    MS = dims["MS"]
    kpos = dims["kpos"]
    PW = dims["PW"]
    Q = dims["Q"]
    NPq = NP // Q
    AX = mybir.AxisListType
    colmap = dims["colmap"]
    HP1 = H + 1
    CA = 132              # atom payload row: [hs(128) | s | pad3]
    CM = 264              # mol payload row: [xcur(128) | hs(128) | s | pad7]
    assert 0 < kpos < P, f"degenerate att_l sign split: kpos={kpos}"

    nc = bacc.Bacc("TRN2", target_bir_lowering=False, debug=False, num_devices=R)

    xT_d = nc.dram_tensor("xT", [NODE_IN, NP], F32, kind="ExternalInput")
    eaT_d = nc.dram_tensor("eaT", [EDGE_IN, ES], F32, kind="ExternalInput")
    srcg_d = nc.dram_tensor("src_gid", [P, EC], I32, kind="ExternalInput")
    dcode_d = nc.dram_tensor("dcode", [P, EC], F32, kind="ExternalInput")
    mnidx_d = nc.dram_tensor("mol_nidx", [P, MSC], I32, kind="ExternalInput")
    mcode_d = nc.dram_tensor("mcode", [P, MSC], F32, kind="ExternalInput")
    wpack_d = nc.dram_tensor("wpack", [P, PW], F32, kind="ExternalInput")
    lin1wT_d = nc.dram_tensor("lin1_wT", [NODE_IN, P], F32, kind="ExternalInput")
    w1efT_d = nc.dram_tensor("w1e_fT", [EDGE_IN, P], F32, kind="ExternalInput")
    out_d = nc.dram_tensor("out", [MP], F32, kind="ExternalOutput")

    def dbgt(name, shape):
        if debug_taps:
            return nc.dram_tensor(name, shape, F32, kind="ExternalOutput")
        return None

    dbg_h0 = dbgt("dbg_h0T", [P, NP])
    dbg_w0 = dbgt("dbg_w0", [P, EC])
    dbg_m0 = dbgt("dbg_m0T", [P, NP])
    dbg_x0 = dbgt("dbg_x0T", [P, NP])
    dbg_x2 = dbgt("dbg_x2T", [P, NP])
    dbg_ro = dbgt("dbg_roT", [P, MP])

    with tile.TileContext(nc) as tc:
        with tc.tile_pool(name="res", bufs=1) as res, \
             tc.tile_pool(name="big", bufs=1) as big, \
             tc.tile_pool(name="stp", bufs=1) as stp, \
             tc.tile_pool(name="sc", bufs=2) as sc, \
             tc.tile_pool(name="wk", bufs=2) as wk, \
             tc.tile_pool(name="gp", bufs=3) as gp, \
             tc.tile_pool(name="molp", bufs=1) as molp, \
             tc.tile_pool(name="dram", bufs=1, space="DRAM") as dram, \
             tc.tile_pool(name="ps2", bufs=2, space="PSUM") as ps2, \
             tc.tile_pool(name="pscat", bufs=2, space="PSUM") as pscat, \
             tc.tile_pool(name="ptp", bufs=2, space="PSUM") as ptp, \
             tc.tile_pool(name="psml", bufs=1, space="PSUM") as psml:

            # ---------------- resident constants ----------------
            ident = res.tile([P, P], F32)
            make_identity(nc, ident[:])
            ones_row = res.tile([1, P], F32)
            nc.gpsimd.memset(ones_row[:], 1.0)
            iota_i = res.tile([P, T * P], I32)
            nc.gpsimd.iota(iota_i[:], pattern=[[0, T], [1, P]], base=0,
                           channel_multiplier=0)
            iota_e = res.tile([P, T * P], F32)
            nc.vector.tensor_copy(iota_e[:], iota_i[:])

            wp = res.tile([P, PW], F32)
            nc.sync.dma_start(wp[:], wpack_d[:])
            lin1_wT = res.tile([NODE_IN, P], F32)
            nc.sync.dma_start(lin1_wT[:], lin1wT_d[:])
            w1e_fT = res.tile([EDGE_IN, P], F32)
            nc.sync.dma_start(w1e_fT[:], w1efT_d[:])

            def wcol(name):
                s, n = colmap[name]
                return wp[:, s:s + n]

            srcg = res.tile([P, EC], I32)
            nc.sync.dma_start(srcg[:], srcg_d[:])
            dcf = res.tile([P, EC], F32)
            nc.sync.dma_start(dcf[:], dcode_d[:])

            # ---------------- DRAM buffers ----------------
            def dtile(shape, tg, shared=False):
                return dram.tile(shape, F32, tag=tg, name=tg,
                                 addr_space="Shared" if shared else "Local")

            h0T_dram = dtile([P, NP], "h0T")
            mT_dram = dtile([P, NP], "mT")
            xcA = dtile([P, NP], "xcA")
            xcB = dtile([P, NP], "xcB")
            tab0l = dtile([NP, 2 * H], "tab0l")
            tab0 = dtile([R * NP, 2 * H], "tab0", shared=True)
            tabA_l = [dtile([NP, CA], f"tabA_l{i}") for i in range(NUM_ATOM_EXTRA)]
            tabA = [dtile([R * NP, CA], f"tabA{i}", shared=True)
                    for i in range(NUM_ATOM_EXTRA)]
            mrows = dtile([NP, CM], "mrows")
            mstream = dtile([MS, CA], "mstream")
            tn_dram = dtile([NP, 1], "tn_dram")

            chunks = [(cs, min(512, NP - cs)) for cs in range(0, NP, 512)]
            mol_chunks = [(cs, min(512, MP - cs)) for cs in range(0, MP, 512)]

            def rows_ap(tab, cs, L):
                return tab[cs:cs + L, :].rearrange("(t p) h -> p t h", p=P)

            def s512(tag):
                return sc.tile([P, 512], F32, tag=tag, name=tag)

            rg = [list(range(R))]

            def ag_quarters(tab_l, tab_g, done, state):
                """Emit per-quarter AllGathers as staging rows complete."""
                while state[0] < Q and done >= (state[0] + 1) * NPq:
                    q = state[0]
                    nc.gpsimd.collective_compute(
                        "AllGather", OP.bypass, replica_groups=rg,
                        ins=[tab_l[q * NPq:(q + 1) * NPq, :].opt()],
                        outs=[tab_g[q * R * NPq:(q + 1) * R * NPq, :].opt()])
                    state[0] += 1

            # =========================================================
            # GATE node phase -> tab0 rows ([p1 | g2]) + t0 staging
            # =========================================================
            t0stag = stp.tile([P, W], F32, tag="t0stag")
            agst0 = [0]
            for cs, L in chunks:
                nt = L // P
                xin = sc.tile([NODE_IN, 512], F32, tag="xin")
                nc.sync.dma_start(xin[:, :L], xT_d[:, cs:cs + L])
                pm = ps2.tile([P, 512], F32, tag="mm512")
                nc.tensor.matmul(out=pm[:, :L], lhsT=lin1_wT[:], rhs=xin[:, :L],
                                 start=True, stop=True)
                h0sb = s512("t1")
                nc.scalar.activation(h0sb[:, :L], pm[:, :L], AF.Lrelu,
                                     bias=wcol("lin1_b"), alpha=0.01)
                nc.sync.dma_start(h0T_dram[:, cs:cs + L], h0sb[:, :L])
                st = wk.tile([P, 4 * 2 * H], F32, tag="rstag", name="rstag0")
                for nm, wname, off in (("p1", "w1x_fT", 0), ("g2", "gate_lin2T", H)):
                    pm2 = ps2.tile([P, 512], F32, tag="mm512")
                    nc.tensor.matmul(out=pm2[:, :L], lhsT=wcol(wname),
                                     rhs=h0sb[:, :L], start=True, stop=True)
                    psb = s512("t2")
                    nc.scalar.activation(psb[:, :L], pm2[:, :L], AF.Copy)
                    for t in range(nt):
                        tp = ptp.tile([P, P], F32, tag="tp")
                        nc.tensor.transpose(tp[:], psb[:, t * P:(t + 1) * P], ident[:])
                        nc.vector.tensor_copy(
                            st[:, t * 2 * H + off:t * 2 * H + off + H], tp[:])
                for t in range(nt):
                    w = cs // P + t
                    pt = psml.tile([P, 2], F32, tag="small")
                    nc.tensor.matmul(out=pt[:, 0:1], lhsT=h0sb[:, t * P:(t + 1) * P],
                                     rhs=wcol("gate_att_r"), start=True, stop=True)
                    nc.vector.tensor_copy(t0stag[:, w:w + 1], pt[:, 0:1])
                nc.sync.dma_start(rows_ap(tab0l, cs, L),
                                  st[:, :nt * 2 * H].rearrange(
                                      "p (t h) -> p t h", h=2 * H))
            if phases >= 2:
                ag_quarters(tab0l, tab0, NP, agst0)
            if dbg_h0 is not None:
                for cs, L in chunks:
                    tmp = s512("t3")
                    nc.sync.dma_start(tmp[:, :L], h0T_dram[:, cs:cs + L])
                    nc.sync.dma_start(dbg_h0[:, cs:cs + L], tmp[:, :L])

            # =========================================================
            # shared helpers
            # =========================================================
            TB = 10
            tm_state = [None]

            def t_stage(tstag_t, nw):
                """Stage per-node t values to DRAM in node order; blocks of TB
                windows are loaded back as [1, TB*P] rows."""
                nc.sync.dma_start(
                    tn_dram[:nw * P, :].rearrange("(w p) one -> p (w one)", p=P),
                    tstag_t[:, :nw])
                return tn_dram

            def tmat_for(tnd, w, nw):
                """T_mat[p, n] = t(window-w node n) for every p — K=1
                broadcast matmul from the block-loaded t row."""
                if w % TB == 0:
                    trb = stp.tile([1, TB * P], F32, tag="tn_row", name="tn_row")
                    hi = min((w + TB) * P, nw * P)
                    nc.sync.dma_start(trb[:, :hi - w * P],
                                      tnd[w * P:hi, :].rearrange("n one -> one n"))
                    tm_state[0] = trb
                wo = w % TB
                tm = ptp.tile([P, P], F32, tag="tmat", name="tmat", bufs=1)
                nc.tensor.matmul(out=tm[:], lhsT=ones_row[:],
                                 rhs=tm_state[0][:, wo * P:(wo + 1) * P],
                                 start=True, stop=True)
                tmsb = wk.tile([P, P], F32, tag="tmsb", name="tmsb")
                nc.vector.tensor_copy(tmsb[:], tm[:])
                return tmsb

            def te_reduce(onehot, tmsb, te_out, tcount):
                """te[p, t] = t(dst of edge lane p in tile t): one batched
                onehot*T_mat multiply + free-dim reduce on the vector engine."""
                scrB = wk.tile([P, T * P], F32, tag="scrB", name="scrB")
                s3 = scrB[:, :tcount * P].rearrange("p (t n) -> p t n", t=tcount)
                nc.vector.tensor_tensor(
                    out=s3,
                    in0=onehot[:, :tcount * P].rearrange("p (t n) -> p t n",
                                                         t=tcount),
                    in1=tmsb[:].rearrange("p (one n) -> p one n",
                                          one=1).to_broadcast([P, tcount, P]),
                    op=OP.mult)
                nc.vector.tensor_reduce(out=te_out, in_=s3, axis=AX.X,
                                        op=OP.add)

            def onehot_for(codes, c0, tcount, iota):
                onehot = wk.tile([P, T * P], F32, tag="onehot", name="onehot")
                nc.vector.tensor_tensor(
                    out=onehot[:, :tcount * P].rearrange("p (t n) -> p t n",
                                                         t=tcount),
                    in0=codes[:, c0:c0 + tcount].to_broadcast([P, tcount, P]),
                    in1=iota[:, :tcount * P].rearrange("p (t n) -> p t n",
                                                       t=tcount),
                    op=OP.is_equal)
                return onehot

            def weight_and_scatter(gwin, stride, moff, wx, ws, onehot, tcount):
                rhs = wk.tile([P, T * HP1], F32, tag="rhs")
                r3 = rhs[:, :tcount * HP1].rearrange("p (t c) -> p t c", t=tcount)
                nc.vector.tensor_tensor(
                    out=r3[:, :, 0:H],
                    in0=gwin[:, :tcount * stride].rearrange(
                        "p (t c) -> p t c", t=tcount)[:, :, moff:moff + H],
                    in1=wx[:, ws].to_broadcast([P, tcount, H]), op=OP.mult)
                nc.vector.tensor_copy(r3[:, :, H:HP1],
                                      wx[:, ws].to_broadcast([P, tcount, 1]))
                return rhs

            def scat_epilogue(psum):
                den = wk.tile([P, 1], F32, tag="den")
                nc.vector.tensor_scalar_add(den[:], psum[:, H:HP1], 1e-16)
                rec = wk.tile([P, 1], F32, tag="rec")
                nc.vector.reciprocal(rec[:], den[:])
                msc = wk.tile([P, H], F32, tag="msc")
                nc.scalar.activation(msc[:], psum[:, 0:H], AF.Copy, scale=rec[:])
                tp = ptp.tile([P, P], F32, tag="tp", name="tp_e")
                nc.tensor.transpose(tp[:], msc[:], ident[:])
                return tp

            # =========================================================
            # GATE edge phase (single pass) -> mT_dram
            # =========================================================
            lp = stp.tile([P, EC], F32, tag="e1", name="lp")
            ln = stp.tile([P, EC], F32, tag="e2", name="ln")
            te = stp.tile([P, EC], F32, tag="e0", name="te")
            wx0 = stp.tile([P, EC], F32, tag="e3", name="wx0")
            tnd0 = t_stage(t0stag, W)
            for w in range(W if phases >= 3 else 0):
                gwin = gp.tile([P, T * 2 * H], F32, tag="gath", name="gath")
                for g in range(T):
                    nc.gpsimd.indirect_dma_start(
                        out=gwin[:, g * 2 * H:(g + 1) * 2 * H], out_offset=None,
                        in_=tab0[:],
                        in_offset=bass.IndirectOffsetOnAxis(
                            ap=srcg[:, w * T + g:w * T + g + 1], axis=0))
                ea_w = gp.tile([EDGE_IN, EW], F32, tag="ea_w")
                nc.sync.dma_start(ea_w[:], eaT_d[:, w * EW:(w + 1) * EW])
                onehot = onehot_for(dcf, w * T, T, iota_e)
                tm = tmat_for(tnd0, w, W)
                ws = slice(w * T, (w + 1) * T)
                xjb = wk.tile([P, T * P], F32, tag="xjbuf", name="xjbuf")
                for g in range(T):
                    pq = ptp.tile([P, P], F32, tag="tp", name="tp_q")
                    nc.tensor.matmul(out=pq[:], lhsT=ea_w[:, g * P:(g + 1) * P],
                                     rhs=w1e_fT[:], start=True, stop=True)
                    nc.vector.tensor_add(xjb[:, g * P:(g + 1) * P], pq[:],
                                         gwin[:, g * 2 * H:g * 2 * H + H])
                nc.scalar.activation(xjb[:], xjb[:], AF.Lrelu, alpha=0.01)
                xj3 = xjb[:].rearrange("p (t h) -> p t h", t=T)
                nc.vector.tensor_reduce(out=lp[:, ws], in_=xj3[:, :, 0:kpos],
                                        axis=AX.X, op=OP.add)
                nc.vector.tensor_reduce(out=ln[:, ws], in_=xj3[:, :, kpos:],
                                        axis=AX.X, op=OP.add)
                te_reduce(onehot, tm, te[:, ws], T)
                nc.vector.tensor_tensor(out=wx0[:, ws], in0=lp[:, ws],
                                        in1=ln[:, ws], op=OP.subtract)
                nc.vector.tensor_tensor(out=wx0[:, ws], in0=wx0[:, ws],
                                        in1=te[:, ws], op=OP.add)
                nc.scalar.activation(wx0[:, ws], wx0[:, ws], AF.Lrelu, alpha=0.01)
                nc.scalar.activation(wx0[:, ws], wx0[:, ws], AF.Exp)
                rhs = weight_and_scatter(gwin, 2 * H, H, wx0, ws, onehot, T)
                psum = pscat.tile([P, HP1], F32, tag="scat")
                for g in range(T):
                    nc.tensor.matmul(out=psum[:],
                                     lhsT=onehot[:, g * P:(g + 1) * P],
                                     rhs=rhs[:, g * HP1:(g + 1) * HP1],
                                     start=(g == 0), stop=(g == T - 1))
                tp = scat_epilogue(psum)
                mloc = wk.tile([P, P], F32, tag="mloc")
                nc.vector.tensor_copy(mloc[:], tp[:])
                nc.sync.dma_start(mT_dram[:, w * P:(w + 1) * P], mloc[:])
            if dbg_w0 is not None and phases >= 3:
                nc.sync.dma_start(dbg_w0[:], wx0[:])
            if dbg_m0 is not None and phases >= 3:
                for cs, L in chunks:
                    tmp = s512("t3")
                    nc.sync.dma_start(tmp[:, :L], mT_dram[:, cs:cs + L])
                    nc.sync.dma_start(dbg_m0[:, cs:cs + L], tmp[:, :L])

            # =========================================================
            # GRU (chunked over columns)
            # =========================================================
            def gru_chunk(pref, bias_name, m_sb, h_t, L, emit):
                bias = wcol(bias_name)
                t1 = s512("t1")
                nc.vector.tensor_scalar(out=t1[:, :L], in0=m_sb[:, :L],
                                        scalar1=bias, scalar2=0.0,
                                        op0=OP.add, op1=OP.min)
                t2 = s512("t2")
                nc.vector.tensor_scalar(out=t2[:, :L], in0=m_sb[:, :L],
                                        scalar1=bias, scalar2=0.0,
                                        op0=OP.add, op1=OP.max)
                t3 = s512("t3")
                nc.scalar.activation(t3[:, :L], t1[:, :L], AF.Exp)
                q = s512("q")
                nc.vector.tensor_add(q[:, :L], t3[:, :L], t2[:, :L])
                gate_sb = {}
                for gname in ("r", "z"):
                    pg = ps2.tile([P, 512], F32, tag="mm512")
                    nc.tensor.matmul(out=pg[:, :L],
                                     lhsT=wcol(f"{pref}_wih_{gname}"),
                                     rhs=q[:, :L], start=True, stop=False)
                    nc.tensor.matmul(out=pg[:, :L],
                                     lhsT=wcol(f"{pref}_whh_{gname}"),
                                     rhs=h_t[:, :L], start=False, stop=True)
                    gsb = s512(f"g{gname}")
                    nc.scalar.activation(gsb[:, :L], pg[:, :L], AF.Sigmoid,
                                         bias=wcol(f"{pref}_b_{gname}"))
                    gate_sb[gname] = gsb
                pn1 = ps2.tile([P, 512], F32, tag="mm512")
                nc.tensor.matmul(out=pn1[:, :L], lhsT=wcol(f"{pref}_wih_n"),
                                 rhs=q[:, :L], start=True, stop=True)
                pn2 = ps2.tile([P, 512], F32, tag="mm512")
                nc.tensor.matmul(out=pn2[:, :L], lhsT=wcol(f"{pref}_whh_n"),
                                 rhs=h_t[:, :L], start=True, stop=True)
                hnb = s512("hnb")
                nc.vector.tensor_scalar_add(hnb[:, :L], pn2[:, :L],
                                            wcol(f"{pref}_bhh_n"))
                nc.vector.tensor_tensor(out=hnb[:, :L], in0=gate_sb["r"][:, :L],
                                        in1=hnb[:, :L], op=OP.mult)
                nc.vector.tensor_tensor(out=hnb[:, :L], in0=pn1[:, :L],
                                        in1=hnb[:, :L], op=OP.add)
                n_sb = s512("n_sb")
                nc.scalar.activation(n_sb[:, :L], hnb[:, :L], AF.Tanh,
                                     bias=wcol(f"{pref}_bih_n"))
                d_sb = s512("d_sb")
                nc.vector.tensor_tensor(out=d_sb[:, :L], in0=h_t[:, :L],
                                        in1=n_sb[:, :L], op=OP.subtract)
                nc.vector.tensor_tensor(out=d_sb[:, :L], in0=gate_sb["z"][:, :L],
                                        in1=d_sb[:, :L], op=OP.mult)
                nc.vector.tensor_tensor(out=d_sb[:, :L], in0=n_sb[:, :L],
                                        in1=d_sb[:, :L], op=OP.add)
                xo = s512("xo")
                nc.scalar.activation(xo[:, :L], d_sb[:, :L], AF.Relu)
                emit(xo)

            def gru_phase(pref, bias_name, h_dram, xout_dram):
                for cs, L in chunks:
                    m_sb = s512("m_sb")
                    nc.sync.dma_start(m_sb[:, :L], mT_dram[:, cs:cs + L])
                    h_t = s512("h_t")
                    nc.sync.dma_start(h_t[:, :L], h_dram[:, cs:cs + L])
                    gru_chunk(pref, bias_name, m_sb, h_t, L,
                              lambda xo, cs=cs, L=L: nc.sync.dma_start(
                                  xout_dram[:, cs:cs + L], xo[:, :L]))

            if phases >= 4:
                gru_phase("gru0", "gate_bias", h0T_dram, xcA)
            if dbg_x0 is not None and phases >= 4:
                for cs, L in chunks:
                    tmp = s512("t3")
                    nc.sync.dma_start(tmp[:, :L], xcA[:, cs:cs + L])
                    nc.sync.dma_start(dbg_x0[:, cs:cs + L], tmp[:, :L])

            # =========================================================
            # Atom layers
            # =========================================================
            xin_t, xout_t = xcA, xcB
            agstA = [[0] for _ in range(NUM_ATOM_EXTRA)]
            for l in range(NUM_ATOM_EXTRA if phases >= 5 else 0):
                tstag_a = stp.tile([P, W], F32, tag="t0stag", name="tstag_a")
                for cs, L in chunks:
                    nt = L // P
                    xc_sb = s512("m_sb")
                    nc.sync.dma_start(xc_sb[:, :L], xin_t[:, cs:cs + L])
                    pm = ps2.tile([P, 512], F32, tag="mm512")
                    nc.tensor.matmul(out=pm[:, :L], lhsT=wcol(f"atom{l}_linT"),
                                     rhs=xc_sb[:, :L], start=True, stop=True)
                    hs_sb = s512("t1")
                    nc.scalar.activation(hs_sb[:, :L], pm[:, :L], AF.Copy)
                    st = wk.tile([P, 4 * CA], F32, tag="rstag", name="rstagA")
                    for t in range(nt):
                        w = cs // P + t
                        tp = ptp.tile([P, P], F32, tag="tp")
                        nc.tensor.transpose(tp[:], hs_sb[:, t * P:(t + 1) * P],
                                            ident[:])
                        nc.vector.tensor_copy(st[:, t * CA:t * CA + H], tp[:])
                        pt = psml.tile([P, 2], F32, tag="small")
                        nc.tensor.matmul(out=pt[:],
                                         lhsT=hs_sb[:, t * P:(t + 1) * P],
                                         rhs=wcol(f"atom{l}_att"), start=True,
                                         stop=True)
                        nc.vector.tensor_copy(st[:, t * CA + H:t * CA + H + 1],
                                              pt[:, 0:1])
                        nc.vector.tensor_copy(tstag_a[:, w:w + 1], pt[:, 1:2])
                    nc.sync.dma_start(rows_ap(tabA_l[l], cs, L),
                                      st[:, :nt * CA].rearrange(
                                          "p (t h) -> p t h", h=CA))
                ag_quarters(tabA_l[l], tabA[l], NP, agstA[l])

                wxa = stp.tile([P, EC], F32, tag="e3", name="wxa")
                tnda = t_stage(tstag_a, W)
                for w in range(W):
                    gwin = gp.tile([P, T * CA], F32, tag="gath", name="gath")
                    for g in range(T):
                        nc.gpsimd.indirect_dma_start(
                            out=gwin[:, g * CA:(g + 1) * CA], out_offset=None,
                            in_=tabA[l][:],
                            in_offset=bass.IndirectOffsetOnAxis(
                                ap=srcg[:, w * T + g:w * T + g + 1], axis=0))
                    onehot = onehot_for(dcf, w * T, T, iota_e)
                    tm = tmat_for(tnda, w, W)
                    ws = slice(w * T, (w + 1) * T)
                    te_reduce(onehot, tm, te[:, ws], T)
                    # lp cols <- s from payload (strided copy, all T tiles)
                    nc.vector.tensor_copy(
                        lp[:, ws],
                        gwin[:, :T * CA].rearrange(
                            "p (t c) -> p t c", t=T)[:, :, H:H + 1].rearrange(
                            "p t one -> p (t one)"))
                    nc.vector.tensor_tensor(out=wxa[:, ws], in0=lp[:, ws],
                                            in1=te[:, ws], op=OP.add)
                    nc.scalar.activation(wxa[:, ws], wxa[:, ws], AF.Lrelu,
                                         alpha=0.01)
                    nc.scalar.activation(wxa[:, ws], wxa[:, ws], AF.Exp)
                    rhs = weight_and_scatter(gwin, CA, 0, wxa, ws, onehot, T)
                    psum = pscat.tile([P, HP1], F32, tag="scat")
                    for g in range(T):
                        nc.tensor.matmul(out=psum[:],
                                         lhsT=onehot[:, g * P:(g + 1) * P],
                                         rhs=rhs[:, g * HP1:(g + 1) * HP1],
                                         start=(g == 0), stop=(g == T - 1))
                    tp = scat_epilogue(psum)
                    mloc = wk.tile([P, P], F32, tag="mloc")
                    nc.vector.tensor_copy(mloc[:], tp[:])
                    nc.sync.dma_start(mT_dram[:, w * P:(w + 1) * P], mloc[:])

                gru_phase(f"atom{l}", f"atom{l}_bias", xin_t, xout_t)
                xin_t, xout_t = xout_t, xin_t
            xcur_d = xin_t
            if dbg_x2 is not None and phases >= 5:
                for cs, L in chunks:
                    tmp = s512("t3")
                    nc.sync.dma_start(tmp[:, :L], xcur_d[:, cs:cs + L])
                    nc.sync.dma_start(dbg_x2[:, cs:cs + L], tmp[:, :L])

            # =========================================================
            # Mol phase
            # =========================================================
            outT = big.tile([P, MP], F32, tag="outT")
            if phases < 6:
                nc.gpsimd.memset(outT[:], 0.0)
            mni = res.tile([P, MSC], I32)
            nc.sync.dma_start(mni[:], mnidx_d[:])
            mcf = res.tile([P, MSC], F32)
            nc.sync.dma_start(mcf[:], mcode_d[:])

            ngr = (TNt + T - 1) // T
            for cs, L in (chunks if phases >= 6 else []):
                nt = L // P
                xc_sb = s512("m_sb")
                nc.sync.dma_start(xc_sb[:, :L], xcur_d[:, cs:cs + L])
                pm = ps2.tile([P, 512], F32, tag="mm512")
                nc.tensor.matmul(out=pm[:, :L], lhsT=wcol("mol_linT"),
                                 rhs=xc_sb[:, :L], start=True, stop=True)
                hs_sb = s512("t1")
                nc.scalar.activation(hs_sb[:, :L], pm[:, :L], AF.Copy)
                st = wk.tile([P, 4 * CM], F32, tag="rstag", name="rstagM")
                for t in range(nt):
                    tp = ptp.tile([P, P], F32, tag="tp")
                    nc.tensor.transpose(tp[:], xc_sb[:, t * P:(t + 1) * P],
                                        ident[:])
                    nc.vector.tensor_copy(st[:, t * CM:t * CM + H], tp[:])
                    tp2 = ptp.tile([P, P], F32, tag="tp")
                    nc.tensor.transpose(tp2[:], hs_sb[:, t * P:(t + 1) * P],
                                        ident[:])
                    nc.vector.tensor_copy(st[:, t * CM + H:t * CM + 2 * H], tp2[:])
                    pt = psml.tile([P, 2], F32, tag="small")
                    nc.tensor.matmul(out=pt[:, 0:1],
                                     lhsT=hs_sb[:, t * P:(t + 1) * P],
                                     rhs=wcol("mol_att_src"), start=True,
                                     stop=True)
                    nc.vector.tensor_copy(
                        st[:, t * CM + 2 * H:t * CM + 2 * H + 1], pt[:, 0:1])
                nc.sync.dma_start(
                    mrows[cs:cs + L, :].rearrange("(t p) h -> p t h", p=P),
                    st[:, :nt * CM].rearrange("p (t h) -> p t h", h=CM))

            # readout: gather combined rows; scatter xcur part; stage [hs|s]
            for w in range(MW if phases >= 6 else 0):
                psum = pscat.tile([P, HP1], F32, tag="scat", name="mpsum")
                for gi in range(ngr):
                    g0 = gi * T
                    gt = min(T, TNt - g0)
                    c0 = w * TNt + g0
                    gath = gp.tile([P, T * CM], F32, tag="gath", name="gath")
                    for g in range(gt):
                        nc.gpsimd.indirect_dma_start(
                            out=gath[:, g * CM:(g + 1) * CM], out_offset=None,
                            in_=mrows[:],
                            in_offset=bass.IndirectOffsetOnAxis(
                                ap=mni[:, c0 + g:c0 + g + 1], axis=0))
                    nc.sync.dma_start(
                        mstream[c0 * P:(c0 + gt) * P, :].rearrange(
                            "(t p) h -> p t h", p=P),
                        gath[:, :gt * CM].rearrange(
                            "p (t c) -> p t c", t=gt)[:, :, H:H + CA])
                    onehot = onehot_for(mcf, c0, gt, iota_e)
                    for g in range(gt):
                        nc.tensor.matmul(
                            out=psum[:, 0:H],
                            lhsT=onehot[:, g * P:(g + 1) * P],
                            rhs=gath[:, g * CM:g * CM + H],
                            start=(gi == 0 and g == 0),
                            stop=(gi == ngr - 1 and g == gt - 1))
                rsb = wk.tile([P, H], F32, tag="msc")
                nc.scalar.activation(rsb[:], psum[:, 0:H], AF.Relu)
                tp = ptp.tile([P, P], F32, tag="tp", name="tp_m")
                nc.tensor.transpose(tp[:], rsb[:], ident[:])
                nc.vector.tensor_copy(outT[:, w * P:(w + 1) * P], tp[:])
            if dbg_ro is not None:
                nc.sync.dma_start(dbg_ro[:], outT[:])

            mGT = big.tile([P, MP], F32, tag="mGT")
            mtstag = stp.tile([P, W], F32, tag="t0stag", name="mtstag")
            wsm = stp.tile([P, MSC], F32, tag="wsm", name="wsm")
            tem = stp.tile([P, MSC], F32, tag="tem", name="tem")
            for ts in range(NUM_TIMESTEPS if phases >= 6 else 0):
                for cs, L in mol_chunks:
                    pm = ps2.tile([P, 512], F32, tag="mm512")
                    nc.tensor.matmul(out=pm[:, :L], lhsT=wcol("mol_linT"),
                                     rhs=outT[:, cs:cs + L], start=True,
                                     stop=True)
                    hd_sb = s512("t1")
                    nc.scalar.activation(hd_sb[:, :L], pm[:, :L], AF.Copy)
                    for t in range(L // P):
                        w = cs // P + t
                        pt = psml.tile([P, 2], F32, tag="small")
                        nc.tensor.matmul(out=pt[:, 0:1],
                                         lhsT=hd_sb[:, t * P:(t + 1) * P],
                                         rhs=wcol("mol_att_dst"), start=True,
                                         stop=True)
                        nc.vector.tensor_copy(mtstag[:, w:w + 1], pt[:, 0:1])
                tndm = t_stage(mtstag, MW)
                for w in range(MW):
                    tm = tmat_for(tndm, w, MW)
                    c0w = w * TNt
                    wslw = slice(c0w, c0w + TNt)
                    msw = molp.tile([P, TNt * CA], F32, tag="msw", name="msw")
                    nc.sync.dma_start(
                        msw[:],
                        mstream[c0w * P:(c0w + TNt) * P, :].rearrange(
                            "(t p) h -> p t h", p=P))
                    ms3 = msw[:].rearrange("p (t c) -> p t c", t=TNt)
                    # one-hot for the whole window (TNt tiles)
                    ohm = molp.tile([P, TNt * P], F32, tag="ohm", name="ohm")
                    ohm3 = ohm[:].rearrange("p (t n) -> p t n", t=TNt)
                    nc.vector.tensor_tensor(
                        out=ohm3,
                        in0=mcf[:, wslw].to_broadcast([P, TNt, P]),
                        in1=iota_e[:, :P].rearrange(
                            "p (one n) -> p one n",
                            one=1).to_broadcast([P, TNt, P]),
                        op=OP.is_equal)
                    # logits for the whole window (te group-wise into scrB)
                    for gi in range(ngr):
                        g0 = gi * T
                        gt = min(T, TNt - g0)
                        te_reduce(ohm[:, g0 * P:], tm,
                                  tem[:, c0w + g0:c0w + g0 + gt], gt)
                    nc.vector.tensor_copy(
                        wsm[:, wslw],
                        ms3[:, :, H:H + 1].rearrange("p t one -> p (t one)"))
                    nc.vector.tensor_tensor(out=wsm[:, wslw], in0=wsm[:, wslw],
                                            in1=tem[:, wslw], op=OP.add)
                    nc.scalar.activation(wsm[:, wslw], wsm[:, wslw], AF.Lrelu,
                                         alpha=0.01)
                    nc.scalar.activation(wsm[:, wslw], wsm[:, wslw], AF.Exp)
                    psum = pscat.tile([P, HP1], F32, tag="scat", name="mpsum")
                    for gi in range(ngr):
                        g0 = gi * T
                        gt = min(T, TNt - g0)
                        wsl = slice(c0w + g0, c0w + g0 + gt)
                        rhs = weight_and_scatter(
                            msw[:, g0 * CA:], CA, 0, wsm, wsl, None, gt)
                        for g in range(gt):
                            nc.tensor.matmul(
                                out=psum[:],
                                lhsT=ohm[:, (g0 + g) * P:(g0 + g + 1) * P],
                                rhs=rhs[:, g * HP1:(g + 1) * HP1],
                                start=(gi == 0 and g == 0),
                                stop=(gi == ngr - 1 and g == gt - 1))
                    tp = scat_epilogue(psum)
                    nc.vector.tensor_copy(mGT[:, w * P:(w + 1) * P], tp[:])

                for cs, L in mol_chunks:
                    m_sb = s512("m_sb")
                    nc.vector.tensor_copy(m_sb[:, :L], mGT[:, cs:cs + L])
                    h_t = s512("h_t")
                    nc.vector.tensor_copy(h_t[:, :L], outT[:, cs:cs + L])
                    gru_chunk("mol", "mol_bias", m_sb, h_t, L,
                              lambda xo, cs=cs, L=L: nc.vector.tensor_copy(
                                  outT[:, cs:cs + L], xo[:, :L]))

            # =========================================================
            # Final MLP
            # =========================================================
            h1all = molp.tile([P, (OUT_DIM // P) * MP], F32, tag="ohm",
                              name="h1all")
            h1 = [h1all[:, j * MP:(j + 1) * MP] for j in range(OUT_DIM // P)]
            for j in range(OUT_DIM // P):
                for cs, L in mol_chunks:
                    pm = ps2.tile([P, 512], F32, tag="mm512")
                    nc.tensor.matmul(out=pm[:, :L], lhsT=wcol(f"lin2T_{j}"),
                                     rhs=outT[:, cs:cs + L], start=True,
                                     stop=True)
                    nc.scalar.activation(h1[j][:, cs:cs + L], pm[:, :L], AF.Relu,
                                         bias=wcol("lin2_b")[:, j:j + 1])
            h2all = molp.tile([P, (MLP_H // P) * MP], F32, tag="msw",
                              name="h2all")
            h2 = [h2all[:, j * MP:(j + 1) * MP] for j in range(MLP_H // P)]
            for j2 in range(MLP_H // P):
                for cs, L in mol_chunks:
                    pm = ps2.tile([P, 512], F32, tag="mm512")
                    for kc in range(OUT_DIM // P):
                        nc.tensor.matmul(out=pm[:, :L],
                                         lhsT=wcol(f"mlp1T_{j2}_{kc}"),
                                         rhs=h1[kc][:, cs:cs + L],
                                         start=(kc == 0),
                                         stop=(kc == OUT_DIM // P - 1))
                    nc.scalar.activation(h2[j2][:, cs:cs + L], pm[:, :L], AF.Relu,
                                         bias=wcol("mlp1_b")[:, j2:j2 + 1])
            osb = stp.tile([1, MP], F32, tag="osb")
            for cs, L in mol_chunks:
                pm = psml.tile([1, 512], F32, tag="small", name="finps")
                for kc in range(MLP_H // P):
                    nc.tensor.matmul(out=pm[:, :L], lhsT=wcol(f"mlp2T_{kc}"),
                                     rhs=h2[kc][:, cs:cs + L],
                                     start=(kc == 0), stop=(kc == MLP_H // P - 1))
                nc.vector.tensor_scalar_add(osb[:, cs:cs + L], pm[:, :L],
                                            dims["mlp2_b"])
            nc.sync.dma_start(out_d[:].rearrange("(one m) -> one m", one=1),
                              osb[:])

    nc.compile()
    return nc


# ----------------------------------------------------------------------------
# entry point
# ----------------------------------------------------------------------------

def run(inputs, debug_taps=False, trace=False):
    dims, in_maps, _aux = prep(inputs)
    nc = build(dims, debug_taps=debug_taps)
    res = bass_utils.run_bass_kernel_spmd(
        nc, in_maps, core_ids=list(range(R)), trace=trace)
    MPR = dims["MPR"]
    out = np.concatenate([res.results[c]["out"][:MPR].reshape(-1)
                          for c in range(R)])
    return out[:dims["M"]].astype(np.float32), res, dims


def kernel(**inputs):
    out, _, _ = run(inputs)
    return out

